# revision 40
# baseline (speedup 1.0000x reference)
"""Trainium2 Bass kernel for nn_NetFV (NetFV pooling head).

Strategy (pure data parallel over 8 cores, 256 batches each):
  - DMA is fully contiguous (the baseline's strided xg write generated
    153k 120-byte descriptors = 2.7ms of DMA engine time). x arrives as
    [120, 80x64] blocks of (x|1|0,0,0); GPSIMD spreads them into the even
    64-halves of a [120, 80x128] weight tile, DVE/ACT write squares into
    the odd halves. All junk lanes compute to exact zeros.
  - Logits: block-diagonal pairing. Two chunks' x^T stacked on 122
    partitions, rhs = [122, 16] block-diag (W|b), so one matmul yields
    logits for 240 rows -> 640 matmuls/core instead of 1280.
  - fv: weights = (x|1|0|x^2|1|0) [120, 128] FWL blocks, rhs = act
    [120, 8], accumulated over 5 chunks into a [128, 512] PSUM bank per
    64 batches. The ones column makes PSUM partition 60 the act-sum, so
    no separate asum matmuls (baseline spent 1280 of them).
  - Finishing processes fv1 (partitions 0:60) and fv2 (64:124) halves in
    single [124, 512] DVE ops with per-partition folded constants;
    partition reductions/broadcasts via tiny PE matmuls; head emits
    y^T [18, 64] per group, one output DMA per core.
"""

import math
import sys

for _p in ("/opt/trn_rl_repo", "/opt/pypackages"):
    if _p not in sys.path:
        sys.path.append(_p)

import ml_dtypes
import numpy as np

import concourse.bacc as bacc
import concourse.bass as bass
import concourse.mybir as mybir
import concourse.tile as tile
from concourse.bass_utils import run_bass_kernel_spmd

F, M, C, OUT = 60, 600, 8, 18
B = 2048
NCORES = 8
BL = B // NCORES            # 256 batches per core
BPB = 16                    # batches per block
NBLK = BL // BPB            # 16 blocks
CH = 5                      # chunks (of 120 rows) per batch
RP = M // CH                # 120 rows per chunk
FGB = 64                    # batches per finishing group
NFG = BL // FGB             # 4 groups
BPF = FGB // BPB            # 4 blocks per group
XW = 608                    # per-batch transposed window width
NBX = BPB * CH              # 80 blocks per xg tile
SQ_DVE = 40                 # squares: blocks [0, SQ_DVE) on DVE, rest on ACT

BF16 = mybir.dt.bfloat16
F32 = mybir.dt.float32
MULT = mybir.AluOpType.mult
EPS = 1e-12


def _build_nc():
    nc = bacc.Bacc(
        "TRN2", target_bir_lowering=False, debug=False,
        enable_asserts=False, num_devices=NCORES,
    )
    xg = nc.dram_tensor("xg", [NBLK, 128, NBX * 64], BF16,
                        kind="ExternalInput").ap()
    xt = nc.dram_tensor("xt", [NBLK // 2, 128, 16 * XW], BF16,
                        kind="ExternalInput").ap()
    waug_d = nc.dram_tensor("waug", [2 * (F + 1), 16], BF16,
                            kind="ExternalInput").ap()
    cst_d = nc.dram_tensor("cst", [128, 3 * C], F32,
                           kind="ExternalInput").ap()
    hds_d = nc.dram_tensor("hds", [124, C * OUT], F32,
                           kind="ExternalInput").ap()
    pew_d = nc.dram_tensor("pew", [128, 124], F32, kind="ExternalInput").ap()
    posm_d = nc.dram_tensor("posm", [128, 3], F32, kind="ExternalInput").ap()
    pw3_d = nc.dram_tensor("pw3", [1, 128], F32, kind="ExternalInput").ap()
    y = nc.dram_tensor("y", [OUT, BL], F32, kind="ExternalOutput").ap()

    with tile.TileContext(nc) as tc:
        _emit(tc, y, xg, xt, waug_d, cst_d, hds_d, pew_d, posm_d, pw3_d)
    nc.compile()
    return nc


def _emit(tc, y, xg, xt, waug_d, cst_d, hds_d, pew_d, posm_d, pw3_d):
    nc = tc.nc
    from contextlib import ExitStack
    ctx = ExitStack()
    with ctx:
        cpool = ctx.enter_context(tc.tile_pool(name="cpool", bufs=1))
        xspool = ctx.enter_context(tc.tile_pool(name="xspool", bufs=2))
        xpool = ctx.enter_context(tc.tile_pool(name="xpool", bufs=2))
        tpool = ctx.enter_context(tc.tile_pool(name="tpool", bufs=3))
        spool = ctx.enter_context(tc.tile_pool(name="spool", bufs=3))
        gpool = ctx.enter_context(tc.tile_pool(name="gpool", bufs=2))
        fpool = ctx.enter_context(tc.tile_pool(name="fpool", bufs=1))
        ypool = ctx.enter_context(tc.tile_pool(name="ypool", bufs=1))
        lpsum = ctx.enter_context(tc.tile_pool(name="lpsum", bufs=2, space="PSUM"))
        fpsum = ctx.enter_context(tc.tile_pool(name="fpsum", bufs=2, space="PSUM"))
        psA = ctx.enter_context(tc.tile_pool(name="psA", bufs=1, space="PSUM"))
        psB = ctx.enter_context(tc.tile_pool(name="psB", bufs=1, space="PSUM"))

        # ---- constants ----
        waug2 = cpool.tile([2 * (F + 1), 16], BF16)
        nc.sync.dma_start(out=waug2[:], in_=waug_d[:])
        cstc = cpool.tile([128, 3 * C], F32)
        nc.sync.dma_start(out=cstc[:], in_=cst_d[:])
        cst = cpool.tile([128, 3 * FGB * C], F32)
        for k3 in range(3):
            nc.scalar.copy(
                cst.rearrange("p (k g e) -> p k g e", k=3, e=C)[:, k3],
                cstc[:, k3 * C:(k3 + 1) * C].unsqueeze(1)
                    .broadcast_to([128, FGB, C]),
            )
        hds = cpool.tile([124, C * OUT], F32)
        nc.sync.dma_start(out=hds[:], in_=hds_d[:])

        pew = cpool.tile([128, 124], F32)    # asum extract+broadcast weights
        nc.sync.dma_start(out=pew[:], in_=pew_d[:])
        posm = cpool.tile([128, 3], F32)     # half-sum + row-select weights
        nc.sync.dma_start(out=posm[:], in_=posm_d[:])
        pw3 = cpool.tile([1, 128], F32)      # norm-broadcast row weights
        nc.sync.dma_start(out=pw3[:], in_=pw3_d[:])
        eps1 = cpool.tile([1, 1], F32)
        nc.vector.memset(eps1[:], EPS)
        yts = ypool.tile([OUT, BL], F32)

        PB = NBX * 64                       # per-block xg columns
        xbig = ypool.tile([128, 4 * PB], BF16)  # rotating 4-block xg buffer
        for fg in range(NFG):
            fp = fpsum.tile([128, FGB * C], F32)
            for b4 in range(BPF):
                blk = fg * BPF + b4
                # ---- load (both HWDGE rings) ----
                if blk % 2 == 0:
                    pair = blk // 2
                    xt_eng = nc.scalar if pair % 2 == 0 else nc.sync
                    xtt = tpool.tile([128, 16 * XW], BF16)
                    xt_eng.dma_start(out=xtt[:], in_=xt[pair])
                xg_eng = nc.sync if blk % 2 == 0 else nc.scalar
                xgo = (blk % 4) * PB
                xg_eng.dma_start(out=xbig[:, xgo:xgo + PB], in_=xg[blk])
                # ---- build fv weight tile ----
                xgh = xbig[0:RP, xgo:xgo + PB]
                xgt = xpool.tile([RP, NBX * 128], BF16)
                xgv = xgt.rearrange("p (k q) -> p k q", q=128)
                xsv = xgh.rearrange("p (k q) -> p k q", q=64)
                nc.vector.tensor_copy(xgv[:, :, 0:64], xsv[:, :, :])
                nc.vector.tensor_tensor(
                    out=xgv[:, 0:SQ_DVE, 64:128],
                    in0=xsv[:, 0:SQ_DVE, :], in1=xsv[:, 0:SQ_DVE, :], op=MULT,
                )
                nc.scalar.square(
                    xgv[:, SQ_DVE:NBX, 64:128], xsv[:, SQ_DVE:NBX, :]
                )
                xto = (blk % 2) * 8 * XW

                # ---- logits: 40 block-diag pair matmuls ----
                lp = lpsum.tile([128, 40 * 16], F32)
                for h in range(2):
                    for g in range(4):
                        for c in range(CH):
                            off = xto + (h * 4 + g) * XW + c * RP
                            p = (h * 4 + g) * CH + c
                            nc.tensor.matmul(
                                lp[:, p * 16:(p + 1) * 16],
                                xtt[0:2 * (F + 1), off:off + 128],
                                waug2[:],
                                start=True, stop=True,
                            )
                # ---- softmax (whole block) ----
                expt = spool.tile([RP, 640], BF16, tag="expt")
                nc.scalar.activation(
                    expt[:], lp[0:RP, :], mybir.ActivationFunctionType.Exp
                )
                sums = spool.tile([RP, 80], F32, tag="sums")
                nc.vector.reduce_sum(
                    out=sums[:],
                    in_=expt.rearrange("p (k e) -> p k e", e=C),
                    axis=mybir.AxisListType.X,
                )
                rin = spool.tile([RP, 80], F32, tag="rin")
                nc.vector.reciprocal_approx_fast(out=rin[:], in_=sums[:])
                rinb = spool.tile([RP, 640], BF16, tag="rinb")
                nc.scalar.copy(
                    rinb.rearrange("p (k e) -> p k e", e=C),
                    rin.unsqueeze(2).broadcast_to([RP, 80, C]),
                )
                actt = spool.tile([RP, 640], BF16, tag="actt")
                nc.vector.tensor_tensor(
                    out=actt[:], in0=expt[:], in1=rinb[:], op=MULT,
                )
                # ---- fv accumulation ----
                for b16 in range(BPB):
                    h, g8 = b16 // 8, b16 % 8
                    pc = (b4 * BPB + b16) * C
                    for c in range(CH):
                        acol = (h * 4 + (g8 % 4)) * CH * 16 + c * 16 \
                            + (g8 // 4) * C
                        nc.tensor.matmul(
                            fp[:, pc:pc + C],
                            xgt[:, (b16 * CH + c) * 128:
                                   (b16 * CH + c + 1) * 128],
                            actt[:, acol:acol + C],
                            start=(c == 0), stop=(c == CH - 1),
                        )

            # ---- finishing, two halves of 32 batches each ----
            HB = FGB // 2
            NG = HB * C
            for fh in range(2):
                fps = fp[:, fh * NG:(fh + 1) * NG]
                stage = gpool.tile([128, NG], F32, tag="stage")
                nc.scalar.copy(stage[:], fps)
                asb = psA.tile([124, NG], F32, tag="pA")
                nc.tensor.matmul(asb[:], pew[:], stage[:],
                                 start=True, stop=True)

                X1 = fpool.tile([124, NG], F32, tag="X1")
                nc.vector.tensor_tensor(out=X1[:], in0=stage[0:124, :],
                                        in1=cst[0:124, 0:NG], op=MULT)
                X2 = fpool.tile([124, NG], F32, tag="X2")
                nc.vector.tensor_tensor(out=X2[:], in0=asb[:],
                                        in1=cst[0:124, 2 * NG:3 * NG],
                                        op=MULT)
                X3 = fpool.tile([124, NG], F32, tag="X3")
                nc.vector.tensor_add(X3[:], X1[:], X2[:])
                X5 = fpool.tile([124, NG], F32, tag="X5")
                nc.vector.tensor_tensor(out=X5[64:64 + F, :],
                                        in0=stage[0:F, :],
                                        in1=cst[0:F, 4 * NG:5 * NG], op=MULT)
                nc.vector.tensor_sub(X3[64:64 + F, :], X3[64:64 + F, :],
                                     X5[64:64 + F, :])
                Q = fpool.tile([124, NG], F32, tag="Q")
                nc.vector.tensor_mul(Q[:], X3[:], X3[:])
                r = psB.tile([2, NG], F32, tag="pB")
                nc.tensor.matmul(r[:], posm[0:124, 0:2], Q[:],
                                 start=True, stop=True)
                rb = fpool.tile([2, NG], F32, tag="rb")
                nc.scalar.copy(rb[:], r[:])
                # fv1 norms: per (batch, cluster) over F
                sqA = fpool.tile([1, NG], F32, tag="sqA")
                nc.scalar.activation(sqA[:], rb[0:1, :],
                                     mybir.ActivationFunctionType.Sqrt,
                                     bias=eps1[:])
                nrA = fpool.tile([1, NG], F32, tag="nrA")
                nc.vector.reciprocal_approx_fast(out=nrA[:], in_=sqA[:])
                # fv2 norm: summed over clusters per batch (row 1 of rb)
                rc2 = fpool.tile([2, HB], F32, tag="rc2")
                nc.vector.reduce_sum(
                    out=rc2[:],
                    in_=rb.rearrange("p (g e) -> p g e", e=C),
                    axis=mybir.AxisListType.X,
                )
                rx2 = psA.tile([1, HB], F32, tag="pA")
                nc.tensor.matmul(rx2[:], posm[0:2, 2:3], rc2[:],
                                 start=True, stop=True)
                rxs = fpool.tile([1, HB], F32, tag="rxs")
                nc.scalar.copy(rxs[:], rx2[:])
                sqB = fpool.tile([1, HB], F32, tag="sqB")
                nc.scalar.activation(sqB[:], rxs[:],
                                     mybir.ActivationFunctionType.Sqrt,
                                     bias=eps1[:])
                nrB = fpool.tile([1, HB], F32, tag="nrB")
                nc.vector.reciprocal_approx_fast(out=nrB[:], in_=sqB[:])
                nrBe = fpool.tile([1, NG], F32, tag="nrBe")
                nc.scalar.copy(
                    nrBe.rearrange("p (g e) -> p g e", e=C),
                    nrB.unsqueeze(2).broadcast_to([1, HB, C]),
                )
                nb = psA.tile([124, NG], F32, tag="pA")
                nc.tensor.matmul(nb[0:64, :], pw3[0:1, 0:64], nrA[:],
                                 start=True, stop=True)
                nc.tensor.matmul(nb[64:124, :], pw3[0:1, 64:124], nrBe[:],
                                 start=True, stop=True)
                fvn = fpool.tile([124, NG], F32, tag="fvn")
                nc.vector.tensor_mul(fvn[:], X3[:], nb[:])
                hp = psB.tile([OUT, HB], F32, tag="pB")
                fvv = fvn.rearrange("p (g e) -> p g e", e=C)
                for ci in range(C):
                    nc.tensor.matmul(
                        hp[:], hds[:, ci * OUT:(ci + 1) * OUT], fvv[:, :, ci],
                        start=(ci == 0), stop=(ci == C - 1),
                    )
                nc.scalar.copy(
                    yts[:, fg * FGB + fh * HB:fg * FGB + (fh + 1) * HB],
                    hp[:],
                )
        nc.sync.dma_start(out=y[:], in_=yts[:])


def _host_prep(reshaped_input, cluster_weights, covar_weights, cluster_biases,
               cluster_weights2, hidden1_weights):
    bf = ml_dtypes.bfloat16
    xb = np.ascontiguousarray(reshaped_input, dtype=np.float32).astype(bf)
    xb = xb.reshape(B, M, F)
    # xg: [cores, NBLK, 120, 80*64] of (x | 1 | 0 0 0) blocks
    x6 = xb.reshape(NCORES, NBLK, BPB, CH, RP, F)
    xgp = np.zeros((NCORES, NBLK, RP, BPB, CH, 64), dtype=bf)
    xgp[..., :F] = x6.transpose(0, 1, 4, 2, 3, 5)
    xgp[..., F] = bf(1.0)
    xgp = xgp.reshape(NCORES, NBLK, RP, NBX * 64)
    xgp128 = np.zeros((NCORES, NBLK, 128, NBX * 64), dtype=bf)
    xgp128[:, :, :RP] = xgp
    xgp = xgp128
    # xt: [cores, NBLK, 122, 8*608]; partitions 0:61 = batches g%8<4 of the
    # block (x^T rows + ones row), 61:122 = batches g%8>=4
    xtr = np.zeros((B, F + 1, XW), dtype=bf)
    xtr[:, :F, :M] = xb.transpose(0, 2, 1)
    xtr[:, F, :M] = bf(1.0)
    x7 = xtr.reshape(NCORES, NBLK, 2, 2, 4, F + 1, XW)
    xtp = (x7.transpose(0, 1, 3, 5, 2, 4, 6)
             .reshape(NCORES, NBLK, 2 * (F + 1), 8 * XW))
    # pad to 128 partitions (multiple-of-8 partition counts spread across
    # all 16 DMA engines; 122 landed on only 2) and pack block pairs
    xtp128 = np.zeros((NCORES, NBLK, 128, 8 * XW), dtype=bf)
    xtp128[:, :, :2 * (F + 1)] = xtp
    xtp = (xtp128.reshape(NCORES, NBLK // 2, 2, 128, 8 * XW)
                 .transpose(0, 1, 3, 2, 4)
                 .reshape(NCORES, NBLK // 2, 128, 16 * XW))

    waug = np.concatenate(
        [cluster_weights, cluster_biases[None, :]], axis=0
    ).astype(bf)                                        # [61, 8]
    waug2 = np.zeros((2 * (F + 1), 16), dtype=bf)
    waug2[:F + 1, :C] = waug
    waug2[F + 1:, C:] = waug

    cw = np.square(covar_weights.astype(np.float64)) + 1e-6       # [F, C]
    w2 = cluster_weights2[0].astype(np.float64)                   # [F, C]
    cstA = np.zeros((128, C))
    cstA[0:F] = 1.0 / cw
    cstA[64:64 + F] = 1.0 / np.square(cw)
    cstB = np.zeros((128, C))
    cstB[0:F] = -w2 / cw
    cstB[64:64 + F] = np.square(w2) / np.square(cw) - 1.0
    cstC = np.zeros((128, C))
    cstC[0:F] = 2.0 * w2 / np.square(cw)
    cst = np.concatenate([cstA, cstB, cstC], axis=1).astype(np.float32)

    h = hidden1_weights.astype(np.float64)              # [2*C*F, OUT]
    h1 = h[:C * F].reshape(F, C, OUT) / math.sqrt(C)    # fold 2nd l2n of fv1
    h2 = h[C * F:].reshape(F, C, OUT)
    hds = np.zeros((124, C * OUT))
    hds[0:F] = h1.reshape(F, C * OUT)
    hds[64:64 + F] = h2.reshape(F, C * OUT)
    hds = hds.astype(np.float32)

    pew = np.zeros((128, 124), dtype=np.float32)
    pew[60, :] = 1.0
    posm = np.zeros((128, 3), dtype=np.float32)
    posm[0:F, 0] = 1.0
    posm[64:64 + F, 1] = 1.0
    posm[1, 2] = 1.0
    pw3 = np.zeros((1, 128), dtype=np.float32)
    pw3[0, 0:F] = 1.0
    pw3[0, 64:64 + F] = 1.0

    in_maps = []
    for ci in range(NCORES):
        in_maps.append({
            "xg": np.ascontiguousarray(xgp[ci]),
            "xt": np.ascontiguousarray(xtp[ci]),
            "waug": waug2,
            "cst": cst,
            "hds": hds,
            "pew": pew,
            "posm": posm,
            "pw3": pw3,
        })
    return in_maps


_CACHE = {}


def _get_nc():
    if "nc" not in _CACHE:
        _CACHE["nc"] = _build_nc()
    return _CACHE["nc"]


def kernel(reshaped_input, cluster_weights, covar_weights, cluster_biases,
           cluster_weights2, hidden1_weights, **_kw):
    in_maps = _host_prep(reshaped_input, cluster_weights, covar_weights,
                         cluster_biases, cluster_weights2, hidden1_weights)
    nc = _get_nc()
    res = run_bass_kernel_spmd(nc, in_maps, list(range(NCORES)))
    ys = [res.results[ci]["y"].T for ci in range(NCORES)]
    return np.ascontiguousarray(np.concatenate(ys, axis=0), dtype=np.float32)


if __name__ == "__main__":
    rng = np.random.default_rng(0)
    fake = {
        "reshaped_input": rng.standard_normal((B * M, F), dtype=np.float32),
        "cluster_weights": rng.standard_normal((F, C)).astype(np.float32) * 0.13,
        "covar_weights": rng.standard_normal((F, C)).astype(np.float32) * 0.13,
        "cluster_biases": rng.standard_normal((C,)).astype(np.float32) * 0.13,
        "cluster_weights2": rng.standard_normal((1, F, C)).astype(np.float32) * 0.13,
        "hidden1_weights": rng.standard_normal((2 * C * F, OUT)).astype(np.float32) * 0.35,
    }
    out = kernel(**fake)
    print("kernel output", out.shape, out.dtype, np.abs(out).mean())
